# revision 1
# baseline (speedup 1.0000x reference)
"""AttnGraphSAGE on 8 Trainium2 NeuronCores (Bass/Tile).

Strategy
--------
Math restructuring: attention logits depend only on the SOURCE node
(att[e,h] = s[src[e],h]), so the segment softmax needs no per-edge work:
with E_h[n] = exp(s[n,h]) (no shift; |s| < ~10 so this is f32-safe),

    alpha[e,h]  = E_h[src] / denom_h[dst],   denom_h[i] = sum_{e->i} E_h[src]
    aggr[i,:]   = sum_h (1/denom_h[i]) * sum_{e->i} E_h[src] * x_jm_h[src]

so the whole edge phase is ONE segment-sum over dst of per-src rows
G[n] = [E_0*x_jm_0 (64f) | E_1*x_jm_1 (64f) | E_0 | E_1]  (130 f32, stored
in 192-f32 / 768B strided rows for dma_gather's 256B-multiple constraint).

Sharding: edges partitioned by dst range (6250 nodes per core, host-sorted
by dst), so segment sums and softmax stats are core-local — no cross-core
reduction of edge data at all.  Per layer: each core computes G rows for
its own nodes (dense matmuls), an AllGather replicates the G table, then
per 128-dst-node block the core dma_gathers the rows of its edges' sources
and reduces them with an indicator matmul on the TensorEngine
(lhsT[e,i] = (dst_local[e]==i), PSUM-accumulated over 128-edge subtiles).
dma_gather's int16 indices only reach 32767, so the G table is addressed
as two views (rows [0,32768) and [32768,N)) and each block's edge list is
host-split into lo/hi runs.  BatchNorm needs one tiny [64,2] AllReduce per
layer.  Everything stays feature-major (transposed) on device; the final
[3,Nc] logits are assembled/transposed on host.
"""
import os
import sys
import time
import types
import hashlib
import contextlib
import ctypes

sys.path.insert(0, "/opt/trn_rl_repo")

import numpy as np

import concourse.bass as bass
import concourse.bacc as bacc
import concourse.mybir as mybir
from concourse import tile, library_config

# ---------------------------------------------------------------- constants
N = 50000
E = 800000
IN = 128
F = 64
H = 2
N_CORES = 8
NC_N = N // N_CORES          # 6250 nodes per core
BLK = 128                    # dst nodes per block
ROW = 192                    # G row stride (f32 elems), 768B
GVAL = 2 * F + H             # 130 used cols
LO_SPLIT = 32768             # int16 index limit for dma_gather
CHUNK = 512                  # phase-A node chunk
F32 = mybir.dt.float32
I16 = mybir.dt.int16
AF = mybir.ActivationFunctionType
OP = mybir.AluOpType
BN_EPS = 1e-5
LEAKY = 0.2


# ------------------------------------------------------- axon profile shim
def _install_hookshim():
    """antenv.axon_hooks is missing in this image; provide it so
    run_bass_kernel_spmd(trace=True) can NTFF-profile."""
    if "antenv.axon_hooks" in sys.modules:
        return
    mod = types.ModuleType("antenv.axon_hooks")
    _h = [None]
    mod.set_axon_ntff_profile_hook = lambda h: _h.__setitem__(0, h)
    mod.get_axon_ntff_profile_hook = lambda: _h[0]
    try:
        import antenv
        sys.modules["antenv.axon_hooks"] = mod
        antenv.axon_hooks = mod
        from trn_agent_boot.trn_boot import _ntff_profile_via_ctypes
        mod.set_axon_ntff_profile_hook(
            _ntff_profile_via_ctypes("/opt/axon/libaxon_pjrt.so")
        )
    except Exception:
        pass


# ------------------------------------------------------------ wait legalize
def legalize_waits(nc):
    """TRN2 TPB instructions have ONE sync-wait slot (EventSemaphore has 2);
    hoist extra waits left by the Tile scheduler into EVSEM prequels."""
    n_fixed = 0
    for func in nc.m.functions:
        for block in func.blocks:
            new_insts = []
            for inst in block.instructions:
                si = inst.sync_info
                waits = list(si.on_wait) if si and si.on_wait else []
                cap = 2 if isinstance(inst, mybir.InstEventSemaphore) else 1
                if isinstance(inst, mybir.InstDrain):
                    cap = 1
                if len(waits) > cap:
                    extra, keep = waits[:-cap], waits[-cap:]
                    for i in range(0, len(extra), 2):
                        new_insts.append(
                            mybir.InstEventSemaphore(
                                name=nc.get_next_instruction_name(),
                                ins=[],
                                outs=[],
                                engine=inst.engine,
                                sync_info=mybir.SyncInfo(
                                    on_wait=extra[i:i + 2], on_update=[]
                                ),
                            )
                        )
                    si.on_wait = keep
                    n_fixed += 1
                new_insts.append(inst)
            block.instructions[:] = new_insts
    return n_fixed


# ----------------------------------------------------------- host preprocess
def preprocess(edge_index, n=N, n_cores=N_CORES, lo_split=LO_SPLIT):
    """Sort edges by dst, partition per core / per 128-dst block, split each
    block's edges into lo/hi source runs, and build the device-ready int16
    index planes + f32 dst-local planes."""
    nc_n = n // n_cores
    nb = (nc_n + BLK - 1) // BLK
    src = np.asarray(edge_index[0], np.int64)
    dst = np.asarray(edge_index[1], np.int64)
    order = np.argsort(dst, kind="stable")
    ds, ss = dst[order], src[order]

    # block id of every (sorted) edge: per-core block index
    core = ds // nc_n
    blk = (ds - core * nc_n) // BLK
    gblk = core * nb + blk                     # global block id, sorted asc
    n_gblk = n_cores * nb
    bbounds = np.searchsorted(gblk, np.arange(n_gblk + 1))

    lo_lists, hi_lists, dl_lists = [], [], []
    s_lo = s_hi = 1
    for g in range(n_gblk):
        e0, e1 = bbounds[g], bbounds[g + 1]
        s_b, d_b = ss[e0:e1], ds[e0:e1]
        base = (g // nb) * nc_n + (g % nb) * BLK
        m = s_b < lo_split
        lo, hi = s_b[m], s_b[~m] - lo_split
        dlo, dhi = d_b[m] - base, d_b[~m] - base
        # ascending source order inside each run -> ascending HBM addresses
        # for the gather's descriptors (row-buffer locality)
        olo, ohi = np.argsort(lo, kind="stable"), np.argsort(hi, kind="stable")
        lo, dlo, hi, dhi = lo[olo], dlo[olo], hi[ohi], dhi[ohi]
        lo_lists.append(lo); hi_lists.append(hi)
        dl_lists.append((dlo, dhi))
        s_lo = max(s_lo, (len(lo) + BLK - 1) // BLK)
        s_hi = max(s_hi, (len(hi) + BLK - 1) // BLK)
    s_max = s_lo + s_hi

    w_call_lo, w_call_hi = s_lo * BLK // 16, s_hi * BLK // 16
    w_idx = nb * (w_call_lo + w_call_hi)
    idx_dev = np.zeros((n_cores, 16, w_idx), np.int16)
    dl_dev = np.full((n_cores, BLK, nb * s_max), -1.0, np.float32)

    def wrap16(vals, width_cols):
        # pad with a valid dummy index (0): dma_gather's ring bookkeeping
        # reserves descriptor space from num_idxs_reg but pushes only
        # un-trimmed indices, so trailing -1 pads corrupt the ring when
        # multiple gathers are queued.  Pad slots are killed by dstloc=-1
        # in the indicator matmul instead.
        k = len(vals)
        a = np.zeros((width_cols * 16,), np.int64)
        a[:k] = vals
        return a.reshape(width_cols, 16).T.astype(np.int16)

    for g in range(n_gblk):
        c, b = g // nb, g % nb
        lo, hi = lo_lists[g], hi_lists[g]
        dlo, dhi = dl_lists[g]
        c0 = b * (w_call_lo + w_call_hi)
        idx_dev[c, :, c0:c0 + w_call_lo] = wrap16(lo, w_call_lo)
        idx_dev[c, :, c0 + w_call_lo:c0 + w_call_lo + w_call_hi] = wrap16(hi, w_call_hi)
        # dst-local per (partition p, subtile s): slot k of a call maps to
        # (p=k%128, s=k//128)
        dcol = np.full((s_max * BLK,), -1.0, np.float32)
        dcol[:len(dlo)] = dlo
        dcol[s_lo * BLK:s_lo * BLK + len(dhi)] = dhi
        dl_dev[c, :, b * s_max:(b + 1) * s_max] = dcol.reshape(s_max, BLK).T

    idx_full = np.tile(idx_dev, (1, 8, 1))     # replicate to 128 partitions
    meta = dict(n=n, n_cores=n_cores, nc_n=nc_n, nb=nb, s_lo=s_lo, s_hi=s_hi,
                s_max=s_max, w_idx=w_idx, lo_split=lo_split,
                w_call_lo=w_call_lo, w_call_hi=w_call_hi)
    return idx_full, dl_dev, meta


def pack_weights(inp):
    """Host-side packing of the small replicated weight tensors."""
    def bd(av):  # [H, 2F] -> block-diag [H*F, H] halves (query, msg)
        av = np.asarray(av, np.float32)
        q = np.zeros((H * F, H), np.float32)
        m = np.zeros((H * F, H), np.float32)
        for h in range(H):
            q[h * F:(h + 1) * F, h] = av[h, :F]
            m[h * F:(h + 1) * F, h] = av[h, F:]
        return q, m

    w = {}
    for l in (0, 1):
        w[f"Wr{l}"] = np.asarray(inp[f"Wr{l}"], np.float32)
        w[f"Wn{l}"] = np.asarray(inp[f"Wn{l}"], np.float32)
        w[f"Wa{l}"] = np.asarray(inp[f"Wa{l}"], np.float32)
        w[f"avq{l}"], w[f"avm{l}"] = bd(inp[f"av{l}"])
        w[f"bn{l}"] = np.stack(
            [np.asarray(inp[f"g{l}"], np.float32),
             np.asarray(inp[f"b{l}"], np.float32)], axis=1)  # [64,2]
    w["headW"] = np.asarray(inp["head_W"], np.float32)       # [64,3]
    w["headb"] = np.asarray(inp["head_b"], np.float32).reshape(3, 1)
    w["iota"] = np.broadcast_to(np.arange(BLK, dtype=np.float32), (BLK, BLK)).copy()
    w["ident"] = np.eye(BLK, dtype=np.float32)
    bo = np.zeros((H, H * F), np.float32)
    for h in range(H):
        bo[h, h * F:(h + 1) * F] = 1.0
    w["blkones"] = bo
    return w


# ------------------------------------------------------------ device program
def build_program(meta):
    nc_n, nb = meta["nc_n"], meta["nb"]
    s_lo, s_hi, s_max = meta["s_lo"], meta["s_hi"], meta["s_max"]
    w_idx = meta["w_idx"]
    w_call_lo, w_call_hi = meta["w_call_lo"], meta["w_call_hi"]
    n = meta["n"]
    n_cores = meta["n_cores"]
    lo_split = meta["lo_split"]
    n_hi = n - lo_split if n > lo_split else 0
    dims = [IN, F]                     # per-layer input dim

    nc = bacc.Bacc(None, num_swdge_queues=4)

    # ---- I/O
    xT = nc.declare_dram_parameter("xT", [IN, nc_n], F32, isOutput=False)
    idx_in = nc.declare_dram_parameter("idx", [BLK, w_idx], I16, isOutput=False)
    dl_in = nc.declare_dram_parameter("dstloc", [BLK, nb * s_max], F32, isOutput=False)
    wext = {}
    for l in (0, 1):
        d = dims[l]
        wext[f"Wr{l}"] = nc.declare_dram_parameter(f"Wr{l}", [d, F], F32, isOutput=False)
        wext[f"Wn{l}"] = nc.declare_dram_parameter(f"Wn{l}", [d, H * F], F32, isOutput=False)
        wext[f"Wa{l}"] = nc.declare_dram_parameter(f"Wa{l}", [d, H * F], F32, isOutput=False)
        wext[f"avq{l}"] = nc.declare_dram_parameter(f"avq{l}", [H * F, H], F32, isOutput=False)
        wext[f"avm{l}"] = nc.declare_dram_parameter(f"avm{l}", [H * F, H], F32, isOutput=False)
        wext[f"bn{l}"] = nc.declare_dram_parameter(f"bn{l}", [F, 2], F32, isOutput=False)
    wext["headW"] = nc.declare_dram_parameter("headW", [F, 3], F32, isOutput=False)
    wext["headb"] = nc.declare_dram_parameter("headb", [3, 1], F32, isOutput=False)
    wext["iota"] = nc.declare_dram_parameter("iota", [BLK, BLK], F32, isOutput=False)
    wext["ident"] = nc.declare_dram_parameter("ident", [BLK, BLK], F32, isOutput=False)
    wext["blkones"] = nc.declare_dram_parameter("blkones", [H, H * F], F32, isOutput=False)
    out_ext = nc.declare_dram_parameter("out", [3, nc_n], F32, isOutput=True)

    # ---- internal DRAM
    g_src = [nc.dram_tensor(f"g_src{l}", [nc_n, ROW], F32) for l in (0, 1)]
    g_full = [nc.dram_tensor(f"g_full{l}", [n, ROW], F32, addr_space="Shared")
              for l in (0, 1)]
    bn_src = [nc.dram_tensor(f"bn_src{l}", [F, 2], F32) for l in (0, 1)]
    bn_out = [nc.dram_tensor(f"bn_out{l}", [F, 2], F32, addr_space="Shared")
              for l in (0, 1)]
    groups = [list(range(n_cores))]

    n_chunks = (nc_n + CHUNK - 1) // CHUNK
    # debug bisect: 1=phaseA, 2=+allgather, 3=+gather, 4=+indmm, 5=+bn, 9=full
    stage_cap = int(os.environ.get("GNN_STAGE", "9"))
    layer_cap = int(os.environ.get("GNN_LAYERS", "2"))

    with tile.TileContext(nc) as tc:
        with contextlib.ExitStack() as ctx:
            cpool = ctx.enter_context(tc.tile_pool(name="const", bufs=1))
            wp = ctx.enter_context(tc.tile_pool(name="work", bufs=2))
            hp = ctx.enter_context(tc.tile_pool(name="resid", bufs=1))
            pp = ctx.enter_context(tc.tile_pool(name="psA", bufs=1, space="PSUM"))
            pb = ctx.enter_context(tc.tile_pool(name="psB", bufs=2, space="PSUM"))

            # ---- load constants
            wsb = {}
            for k, ext in wext.items():
                t = cpool.tile(list(ext.shape), F32, tag=k)
                nc.sync.dma_start(out=t[:], in_=ext[:])
                wsb[k] = t
            idx_sb = cpool.tile([BLK, w_idx], I16, tag="idx")
            nc.sync.dma_start(out=idx_sb[:], in_=idx_in[:])
            dl_sb = cpool.tile([BLK, nb * s_max], F32, tag="dl")
            nc.sync.dma_start(out=dl_sb[:], in_=dl_in[:])

            hT_res = hp.tile([F, nc_n], F32, tag="hres")
            hT_act = hp.tile([F, nc_n], F32, tag="hact")
            nc.vector.memset(hT_act[:], 0.0)
            scr = hp.tile([F, (nc_n + 1) // 2], F32, tag="scr")
            stats = hp.tile([F, 6], F32, tag="stats")
            bnsc = hp.tile([F, 8], F32, tag="bnsc")

            for l in (0, 1)[:layer_cap]:
                d = dims[l]
                # ================= phase A: per-node G rows + x_root =======
                for ci in range(n_chunks):
                    c0 = ci * CHUNK
                    cw = min(CHUNK, nc_n - c0)
                    if l == 0:
                        rhs = wp.tile([IN, CHUNK], F32, tag="xchunk")
                        nc.sync.dma_start(out=rhs[:, :cw], in_=xT[:, c0:c0 + cw])
                        rhs_ap = rhs[:IN, :cw]
                    else:
                        rhs_ap = hT_act[:F, c0:c0 + cw]

                    ps_jm = pp.tile([H * F, CHUNK], F32, tag="jm", space="PSUM")
                    ps_iq = pp.tile([H * F, CHUNK], F32, tag="iq", space="PSUM")
                    ps_r = pp.tile([F, CHUNK], F32, tag="r", space="PSUM")
                    nc.tensor.matmul(out=ps_jm[:, :cw], lhsT=wsb[f"Wn{l}"][:d, :],
                                     rhs=rhs_ap, start=True, stop=True)
                    nc.tensor.matmul(out=ps_iq[:, :cw], lhsT=wsb[f"Wa{l}"][:d, :],
                                     rhs=rhs_ap, start=True, stop=True)
                    nc.tensor.matmul(out=ps_r[:, :cw], lhsT=wsb[f"Wr{l}"][:d, :],
                                     rhs=rhs_ap, start=True, stop=True)
                    # x_root straight into the residual accumulator
                    nc.vector.tensor_copy(hT_res[:, c0:c0 + cw], ps_r[:, :cw])

                    jm = wp.tile([H * F, CHUNK], F32, tag="jm_sb")
                    nc.vector.tensor_copy(jm[:, :cw], ps_jm[:, :cw])
                    # leaky(x) = max(x, 0.2x)
                    lkjm = wp.tile([H * F, CHUNK], F32, tag="lkjm")
                    nc.scalar.mul(lkjm[:, :cw], ps_jm[:, :cw], LEAKY)
                    nc.vector.tensor_tensor(out=lkjm[:, :cw], in0=lkjm[:, :cw],
                                            in1=ps_jm[:, :cw], op=OP.max)
                    lkiq = wp.tile([H * F, CHUNK], F32, tag="lkiq")
                    nc.scalar.mul(lkiq[:, :cw], ps_iq[:, :cw], LEAKY)
                    nc.vector.tensor_tensor(out=lkiq[:, :cw], in0=lkiq[:, :cw],
                                            in1=ps_iq[:, :cw], op=OP.max)
                    ps_s = pp.tile([H, CHUNK], F32, tag="s", space="PSUM")
                    nc.tensor.matmul(out=ps_s[:, :cw], lhsT=wsb[f"avq{l}"][:],
                                     rhs=lkiq[:, :cw], start=True, stop=False)
                    nc.tensor.matmul(out=ps_s[:, :cw], lhsT=wsb[f"avm{l}"][:],
                                     rhs=lkjm[:, :cw], start=False, stop=True)
                    e_sb = wp.tile([H, CHUNK], F32, tag="esb")
                    nc.scalar.activation(e_sb[:, :cw], ps_s[:, :cw], AF.Exp)
                    # reuse the (now dead) iq psum bank for the E broadcast
                    ps_eb = pp.tile([H * F, CHUNK], F32, tag="iq", space="PSUM")
                    nc.tensor.matmul(out=ps_eb[:, :cw], lhsT=wsb["blkones"][:],
                                     rhs=e_sb[:, :cw], start=True, stop=True)
                    y = wp.tile([H * F, CHUNK], F32, tag="y")
                    nc.vector.tensor_tensor(out=y[:, :cw], in0=jm[:, :cw],
                                            in1=ps_eb[:, :cw], op=OP.mult)
                    # write G rows (transpose to node-major)
                    for q in range(0, cw, BLK):
                        qw = min(BLK, cw - q)
                        ps_t = pb.tile([BLK, BLK], F32, tag="tp", space="PSUM")
                        nc.tensor.transpose(out=ps_t[:qw, :], in_=y[:, q:q + qw],
                                            identity=wsb["ident"][:])
                        ps_e = pb.tile([BLK, H], F32, tag="tp", space="PSUM")
                        nc.tensor.transpose(out=ps_e[:qw, :], in_=e_sb[:, q:q + qw],
                                            identity=wsb["ident"][:H, :H])
                        gt = wp.tile([BLK, ROW], F32, tag="gt")
                        nc.vector.tensor_copy(gt[:qw, 0:H * F], ps_t[:qw, :])
                        nc.vector.tensor_copy(gt[:qw, H * F:GVAL], ps_e[:qw, :])
                        nc.vector.memset(gt[:qw, GVAL:ROW], 0.0)
                        nc.sync.dma_start(
                            out=g_src[l][c0 + q:c0 + q + qw, :],
                            in_=gt[:qw, :])

                # ================= AllGather G table =======================
                if stage_cap < 2:
                    continue
                nc.gpsimd.collective_compute(
                    "AllGather", OP.bypass, replica_groups=groups,
                    ins=[g_src[l][:]], outs=[g_full[l][:]])

                # ================= phase B: gather + indicator matmul ======
                if stage_cap < 3:
                    continue
                qn = [0]
                for b in range(nb):
                    b0 = b * BLK
                    bw = min(BLK, nc_n - b0)
                    stage = wp.tile([BLK, s_max, ROW], F32, tag="stage", bufs=3)
                    if b < 3:
                        nc.vector.memset(stage[:], 0.0)
                    c0 = b * (w_call_lo + w_call_hi)

                    # the Q7 gather kernel corrupts its scratch above ~1024
                    # indices per call — cap each call at 8 subtiles
                    def emit_gather(sub0, n_sub, in_view, col0, cap=8):
                        for off in range(0, n_sub, cap):
                            k = min(cap, n_sub - off)
                            nc.gpsimd.dma_gather(
                                out_ap=stage[:, sub0 + off:sub0 + off + k, :],
                                in_ap=in_view,
                                idxs_ap=idx_sb[:, col0 + off * 8:col0 + (off + k) * 8],
                                num_idxs=k * BLK, num_idxs_reg=k * BLK,
                                elem_size=ROW, queue_num=qn[0] % 4)
                            qn[0] += 1

                    emit_gather(0, s_lo, g_full[l][0:lo_split, :], c0)
                    if n_hi > 0:
                        emit_gather(s_lo, s_hi, g_full[l][lo_split:n, :],
                                    c0 + w_call_lo)
                    if stage_cap < 4:
                        continue
                    ind = wp.tile([BLK, s_max * BLK], F32, tag="ind")
                    nc.vector.tensor_tensor(
                        out=ind[:].rearrange("p (s i) -> p s i", i=BLK),
                        in0=dl_sb[:, b * s_max:(b + 1) * s_max][:, :, None]
                            .to_broadcast([BLK, s_max, BLK]),
                        in1=wsb["iota"][:, None, :].to_broadcast([BLK, s_max, BLK]),
                        op=OP.is_equal)
                    ps_blk = pb.tile([BLK, GVAL], F32, tag="blk", space="PSUM")
                    for j in range(s_max):
                        nc.tensor.matmul(out=ps_blk[:],
                                         lhsT=ind[:, j * BLK:(j + 1) * BLK],
                                         rhs=stage[:, j, 0:GVAL],
                                         start=(j == 0), stop=(j == s_max - 1))
                    sb = wp.tile([BLK, GVAL], F32, tag="sbblk")
                    nc.vector.tensor_copy(sb[:], ps_blk[:])
                    rec = wp.tile([BLK, H], F32, tag="rec")
                    nc.vector.tensor_scalar_add(rec[:], sb[:, H * F:GVAL], 1e-30)
                    nc.vector.reciprocal(rec[:], rec[:])
                    agg = wp.tile([BLK, F], F32, tag="agg")
                    tmp = wp.tile([BLK, F], F32, tag="tmp")
                    nc.vector.tensor_tensor(out=agg[:], in0=sb[:, 0:F],
                                            in1=rec[:, 0:1].to_broadcast([BLK, F]),
                                            op=OP.mult)
                    nc.vector.tensor_tensor(out=tmp[:], in0=sb[:, F:2 * F],
                                            in1=rec[:, 1:2].to_broadcast([BLK, F]),
                                            op=OP.mult)
                    nc.vector.tensor_add(out=agg[:], in0=agg[:], in1=tmp[:])
                    ps_t = pb.tile([BLK, BLK], F32, tag="tp", space="PSUM")
                    nc.tensor.transpose(out=ps_t[:F, :], in_=agg[:, :F],
                                        identity=wsb["ident"][:])
                    nc.vector.tensor_add(out=hT_res[:, b0:b0 + bw],
                                         in0=hT_res[:, b0:b0 + bw],
                                         in1=ps_t[:F, :bw])

                # ================= BatchNorm + ReLU ========================
                if stage_cap < 5:
                    continue
                nc.vector.reduce_sum(out=stats[:, 0:1], in_=hT_res[:, 0:nc_n],
                                     axis=mybir.AxisListType.X)
                half = (nc_n + 1) // 2
                nc.scalar.square(scr[:, 0:half], hT_res[:, 0:half])
                nc.vector.reduce_sum(out=stats[:, 1:2], in_=scr[:, 0:half],
                                     axis=mybir.AxisListType.X)
                nc.scalar.square(scr[:, 0:nc_n - half], hT_res[:, half:nc_n])
                nc.vector.reduce_sum(out=stats[:, 4:5], in_=scr[:, 0:nc_n - half],
                                     axis=mybir.AxisListType.X)
                nc.vector.tensor_add(out=stats[:, 1:2], in0=stats[:, 1:2],
                                     in1=stats[:, 4:5])
                nc.sync.dma_start(out=bn_src[l][:], in_=stats[:, 0:2])
                nc.gpsimd.collective_compute(
                    "AllReduce", OP.add, replica_groups=groups,
                    ins=[bn_src[l][:]], outs=[bn_out[l][:]])
                nc.sync.dma_start(out=stats[:, 2:4], in_=bn_out[l][:])
                # mu, meansq, var, scale, shift  (bnsc cols: 0 mu,1 msq,2 var,
                # 3 rec,4 rs,5 scale,6 shift)
                nc.scalar.mul(bnsc[:, 0:1], stats[:, 2:3], 1.0 / n)
                nc.scalar.mul(bnsc[:, 1:2], stats[:, 3:4], 1.0 / n)
                nc.vector.tensor_tensor(out=bnsc[:, 2:3], in0=bnsc[:, 0:1],
                                        in1=bnsc[:, 0:1], op=OP.mult)
                nc.vector.tensor_tensor(out=bnsc[:, 2:3], in0=bnsc[:, 1:2],
                                        in1=bnsc[:, 2:3], op=OP.subtract)
                nc.vector.tensor_scalar_add(bnsc[:, 2:3], bnsc[:, 2:3], BN_EPS)
                nc.vector.reciprocal(bnsc[:, 3:4], bnsc[:, 2:3])
                nc.scalar.sqrt(bnsc[:, 4:5], bnsc[:, 3:4])
                nc.vector.tensor_tensor(out=bnsc[:, 5:6], in0=bnsc[:, 4:5],
                                        in1=wsb[f"bn{l}"][:, 0:1], op=OP.mult)
                nc.vector.tensor_tensor(out=bnsc[:, 6:7], in0=bnsc[:, 0:1],
                                        in1=bnsc[:, 5:6], op=OP.mult)
                nc.vector.tensor_tensor(out=bnsc[:, 6:7], in0=wsb[f"bn{l}"][:, 1:2],
                                        in1=bnsc[:, 6:7], op=OP.subtract)
                nc.scalar.activation(hT_act[:, 0:nc_n], hT_res[:, 0:nc_n],
                                     AF.Relu, bias=bnsc[:, 6:7],
                                     scale=bnsc[:, 5:6])

            # ================= head ========================================
            out_sb = hp.tile([3, nc_n], F32, tag="osb")
            for ci in range(n_chunks):
                c0 = ci * CHUNK
                cw = min(CHUNK, nc_n - c0)
                ps_o = pp.tile([3, CHUNK], F32, tag="s", space="PSUM")
                nc.tensor.matmul(out=ps_o[:, :cw], lhsT=wsb["headW"][:],
                                 rhs=hT_act[:F, c0:c0 + cw], start=True, stop=True)
                nc.scalar.activation(out_sb[:, c0:c0 + cw], ps_o[:, :cw],
                                     AF.Identity, bias=wsb["headb"][:, 0:1])
            nc.sync.dma_start(out=out_ext[:], in_=out_sb[:, 0:nc_n])

    return nc


# ---------------------------------------------------------------- run cache
_CACHE = {}


def _build_inputs(inputs, meta, idx_full, dl_dev):
    w = pack_weights(inputs)
    x = np.asarray(inputs["x"], np.float32)
    nc_n = meta["nc_n"]
    in_maps = []
    for c in range(meta["n_cores"]):
        m = dict(w)
        m["xT"] = np.ascontiguousarray(x[c * nc_n:(c + 1) * nc_n, :].T)
        m["idx"] = np.ascontiguousarray(idx_full[c])
        m["dstloc"] = np.ascontiguousarray(dl_dev[c])
        in_maps.append(m)
    return in_maps


def kernel(**inputs):
    from concourse.bass_utils import run_bass_kernel_spmd

    _install_hookshim()
    edge = np.asarray(inputs["edge_index"])
    key = hashlib.sha1(edge.tobytes()).hexdigest()
    if key not in _CACHE:
        idx_full, dl_dev, meta = preprocess(edge)
        nc = build_program(meta)
        nc.finalize()        # Bacc.compile(): evsems, library loads, regalloc
        n_fix = legalize_waits(nc)
        if n_fix:
            print(f"legalize_waits fixed {n_fix} instructions post-finalize")
        _CACHE[key] = (idx_full, dl_dev, meta, nc)
    idx_full, dl_dev, meta, nc = _CACHE[key]
    in_maps = _build_inputs(inputs, meta, idx_full, dl_dev)
    res = run_bass_kernel_spmd(
        nc, in_maps, list(range(meta["n_cores"])),
        trace=bool(os.environ.get("GNN_TRACE")))
    if res.exec_time_ns is not None:
        print(f"HW exec time: {res.exec_time_ns} ns")
    nc_n = meta["nc_n"]
    out = np.concatenate([res.results[c]["out"] for c in range(meta["n_cores"])],
                         axis=1)  # [3, N]
    return np.ascontiguousarray(out.T).astype(np.float32)

